# revision 15
# baseline (speedup 1.0000x reference)
"""Trainium2 Bass kernel for nn_MBRNNIncrementEstimator (GRU increment estimator).

Model (per batch b):
  X_prior[t] = F^{t+1} x0                       (linear prior scan)
  x_proj     = concat(Y, X_prior) @ W_ih.T + b_ih
  GRU over T with W_hh, b_hh  -> outs
  out        = X_prior + outs @ fc_W.T + fc_b

Sharding: data-parallel over batch B=64 across 8 cores (8 batches/core).
All on-chip compute uses a transposed layout (features on partitions) so
the GRU per-step vector math uses all 128 lanes.

v3 design:
 - inpT (= [Y; X_prior]^T, bf16) and XpT (f32) are built on the host and
   DMA'd straight in: the prior scan is tiny host math, and doing it there
   removes the whole on-device prologue (~150us).
 - Gate pre-activations live entirely in PSUM: each block's bank is
   initialized by ONE K=8 "indicator" matmul broadcasting the per-tile
   biases bank-wide (start=True zeroes the 2KB zero region), then the
   block-level x-projection GEMM and the per-step recurrent matmuls
   accumulate on top (start=False).
 - Per-step burst order r(16), hn(16), z(16): the r and hn groups retire
   early on the tensor engine's per-instruction semaphore channel
   (~34ns/MM), so the latency-critical n-path (t1 = r*hn -> +xn -> tanh)
   starts ~550ns sooner than with hn last.  The z gate is only needed
   after tanh (h' = n + z*(h-n)), so sig_z runs behind tanh on the
   Scalar engine.
 - Block-level fillers (bias init, x-proj GEMM, fc GEMM/transpose/DMA)
   are emitted AFTER each step's chain ops and spread one small chunk
   per step, so the Tile scheduler cannot interleave them into the
   latency-critical matmul burst (which stretched steps 8.1us -> 3.2us).
"""

import os
import numpy as np
import ml_dtypes

B, T, NOBS, MST, HID = 64, 1024, 64, 64, 512
H3 = 3 * HID
NCORES = 8
BS = B // NCORES            # 8 batches per core
C = 8                       # GRU block length (steps per PSUM block)

_compiled = {}
LAST_RESULTS = None


def _build_bass(t_steps):
    import concourse.bass as bass
    import concourse.mybir as mybir
    import concourse.tile as tile
    from concourse import bacc
    from concourse.masks import make_identity

    f32 = mybir.dt.float32
    bf16 = mybir.dt.bfloat16

    Tt = t_steps
    nblk = Tt // C
    NT = Tt * BS

    nc = bacc.Bacc(None, target_bir_lowering=False)
    inpT_d = nc.declare_dram_parameter("inpT", [128, NT], bf16, isOutput=False)
    XpT_d = nc.declare_dram_parameter("XpT", [MST, NT], f32, isOutput=False)
    WihT_d = nc.declare_dram_parameter("WihT", [128, H3], bf16, isOutput=False)
    WhhT_d = nc.declare_dram_parameter("WhhT", [128, 48 * 128], bf16, isOutput=False)
    brz8_d = nc.declare_dram_parameter("brz8", [8, 128], bf16, isOutput=False)
    bnx8_d = nc.declare_dram_parameter("bnx8", [8, 128], bf16, isOutput=False)
    indr_d = nc.declare_dram_parameter("indr", [8, 256], bf16, isOutput=False)
    indz_d = nc.declare_dram_parameter("indz", [8, 256], bf16, isOutput=False)
    indnx_d = nc.declare_dram_parameter("indnx", [8, 512], bf16, isOutput=False)
    fcWT_d = nc.declare_dram_parameter("fcWT", [128, 4 * MST], bf16, isOutput=False)
    fcb_d = nc.declare_dram_parameter("fcb", [MST, 1], f32, isOutput=False)
    out_d = nc.declare_dram_parameter("out", [BS, Tt, MST], f32, isOutput=True)

    with tile.TileContext(nc) as tc:
        with (
            tc.tile_pool(name="singles", bufs=1) as singles,
            tc.tile_pool(name="work", bufs=4) as work,
            tc.tile_pool(name="pgates", bufs=2, space="PSUM") as pgates,
            tc.tile_pool(name="pfc", bufs=1, space="PSUM") as pfc,
        ):
            # ---- resident tensors ----
            wih = singles.tile([128, H3], bf16)
            whh = singles.tile([128, 48 * 128], bf16)
            fcw = singles.tile([128, 4 * MST], bf16)
            brz8 = singles.tile([8, 128], bf16)
            bnx8 = singles.tile([8, 128], bf16)
            indr = singles.tile([8, 256], bf16)
            indz = singles.tile([8, 256], bf16)
            indnx = singles.tile([8, 512], bf16)
            fcb = singles.tile([MST, 1], f32)
            ident = singles.tile([MST, MST], f32)
            inpT = singles.tile([128, NT], bf16)
            XpT = singles.tile([MST, NT], f32)
            outsT = singles.tile([128, (Tt + 1) * 32], bf16)

            nc.sync.dma_start(wih[:], WihT_d[:])
            nc.sync.dma_start(whh[:], WhhT_d[:])
            nc.sync.dma_start(fcw[:], fcWT_d[:])
            nc.sync.dma_start(brz8[:], brz8_d[:])
            nc.sync.dma_start(bnx8[:], bnx8_d[:])
            nc.sync.dma_start(indr[:], indr_d[:])
            nc.sync.dma_start(indz[:], indz_d[:])
            nc.sync.dma_start(indnx[:], indnx_d[:])
            nc.sync.dma_start(fcb[:], fcb_d[:])
            nc.sync.dma_start(inpT[:], inpT_d[:])
            nc.sync.dma_start(XpT[:], XpT_d[:])
            make_identity(nc, ident[:])
            nc.vector.memset(outsT[:, 0:32], 0.0)  # h_0 = 0

            # PSUM layout per block (three banks, double-buffered):
            #   r bank  [128, 256]: col = mi*64 + k*8 + b   (r gate tiles)
            #   z bank  [128, 256]: col = mi*64 + k*8 + b   (z gate tiles)
            #   nx bank [128, 512]: cols 0:256 = hn, 256:512 = xn
            # Separate r/z/nx tiles give precise per-tile hazards: sig_r
            # gates only on the r matmuls, the z matmuls never WAR-stall
            # against sig_r, and sig_z only sees the z matmuls.

            def kslice(ap_full, base, k, ntile):
                return bass.AP(
                    tensor=ap_full.tensor,
                    offset=ap_full.offset + base + k * 8,
                    ap=[list(ap_full.ap[0]), [64, ntile], [1, 8]],
                )

            def hslot(t):
                return outsT[:, t * 32:(t + 1) * 32]

            def ap2d(t, ncol_outer, stride_outer):
                return bass.AP(
                    tensor=t.tensor, offset=t.offset,
                    ap=[list(t.ap[0]), [stride_outer, ncol_outer], [1, 8]],
                )

            def rhs_inp(jb):
                ia = inpT[:]
                return bass.AP(
                    tensor=ia.tensor, offset=ia.offset + jb * C,
                    ap=[list(ia.ap[0]), [1, C], [Tt, BS]],
                )

            def xp_bias(bp, b8, indt, qs):
                # bank init in FD-128 chunks: the first (start=True) zeroes
                # the whole 2KB zero region; a single FD-512 matmul here
                # measures ~625ns and stalls the in-order TE queue.
                for q in qs:
                    nc.tensor.matmul(
                        bp[:, q * 128:(q + 1) * 128], b8[:],
                        indt[:, q * 128:(q + 1) * 128],
                        start=(q == 0), stop=False, skip_group_check=True,
                    )

            def xp_gemm(jb, rp, zp, nxp, ms):
                ri = rhs_inp(jb)
                for m in ms:
                    if m < 4:
                        dst = rp[:, m * 64:(m + 1) * 64]
                    elif m < 8:
                        dst = zp[:, (m - 4) * 64:(m - 3) * 64]
                    else:
                        dst = nxp[:, 256 + (m - 8) * 64:256 + (m - 7) * 64]
                    nc.tensor.matmul(
                        dst, wih[:, m * 128:(m + 1) * 128], ri,
                        start=False, stop=(m >= 8), skip_group_check=True,
                    )

            fc_state = {}

            def fc_mm(jb):
                # TE part only — emitted BEFORE the step's chain ops so the
                # per-tile outsT RAW hazard covers only already-written h
                # slots (emitting after the chain made these matmuls wait
                # on that step's h' at the TE queue head, a ~5us/block
                # cascade).
                t0 = jb * C
                oa = outsT[:]
                psf = pfc.tile([MST, 64], f32, tag="fc")
                for kc in range(4):
                    rhs = bass.AP(
                        tensor=oa.tensor,
                        offset=oa.offset + (t0 + 1) * 32 + kc * 8,
                        ap=[list(oa.ap[0]), [1, BS], [32, C]],
                    )  # (b outer, t inner)
                    nc.tensor.matmul(
                        psf[:], fcw[:, kc * MST:(kc + 1) * MST], rhs,
                        start=(kc == 0), stop=(kc == 3),
                    )
                fc_state["psf"] = psf

            def fc_stt(jb):
                t0 = jb * C
                oT = work.tile([MST, 64], f32, tag="oT")
                xa = XpT[:]
                xp_ap = bass.AP(
                    tensor=xa.tensor, offset=xa.offset + t0,
                    ap=[list(xa.ap[0]), [Tt, BS], [1, C]],
                )
                nc.vector.scalar_tensor_tensor(
                    ap2d(oT, BS, 8), fc_state["psf"][:], fcb[:], xp_ap,
                    op0=mybir.AluOpType.add, op1=mybir.AluOpType.add,
                )
                fc_state["oT"] = oT

            def fc_tr(jb):
                ptr = pfc.tile([MST, 64], f32, tag="tr")
                nc.tensor.transpose(ptr[:], fc_state["oT"][:], ident[:])
                fc_state["ptr"] = ptr

            def fc_copy(jb):
                ot = work.tile([MST, 64], f32, tag="ot")
                nc.vector.tensor_copy(ot[:], fc_state["ptr"][:])
                fc_state["ot"] = ot

            def fc_dma(jb):
                t0 = jb * C
                ot = fc_state["ot"]
                for b in range(BS):
                    nc.sync.dma_start(
                        out_d[b, t0:t0 + C, :], ot[b * 8:(b + 1) * 8, :]
                    )

            Sig = mybir.ActivationFunctionType.Sigmoid
            Tanh = mybir.ActivationFunctionType.Tanh

            r_cur = pgates.tile([128, 256], f32, tag="r")
            z_cur = pgates.tile([128, 256], f32, tag="z")
            nx_cur = pgates.tile([128, 512], f32, tag="nx")
            xp_bias(r_cur, brz8, indr, range(2))
            xp_bias(z_cur, brz8, indz, range(2))
            xp_bias(nx_cur, bnx8, indnx, range(4))
            xp_gemm(0, r_cur, z_cur, nx_cur, range(12))

            for j in range(nblk):
                rp, zp, nxp = r_cur, z_cur, nx_cur
                for k in range(C):
                    t = j * C + k
                    h_rd = hslot(t)
                    last_k = (k == C - 1)

                    # burst: r tiles, hn tiles, z tiles.  With separate
                    # r/z PSUM tiles the hazards are precise: sig_r fires
                    # after the 16 r matmuls retire (~550ns into the burst),
                    # t1 after the hn matmuls (~1100ns), sig_z after the z
                    # matmuls (~1650ns, overlapping the tanh path).
                    for mi in range(4):
                        for kc in range(4):
                            nc.tensor.matmul(
                                rp[:, mi * 64 + k * 8:mi * 64 + k * 8 + 8],
                                whh[:, (kc * 12 + mi) * 128:(kc * 12 + mi + 1) * 128],
                                h_rd[:, kc * 8:(kc + 1) * 8],
                                start=False, stop=(kc == 3),
                                skip_group_check=True,
                            )
                    for i in range(4):
                        for kc in range(4):
                            nc.tensor.matmul(
                                nxp[:, i * 64 + k * 8:i * 64 + k * 8 + 8],
                                whh[:, (kc * 12 + 8 + i) * 128:(kc * 12 + 9 + i) * 128],
                                h_rd[:, kc * 8:(kc + 1) * 8],
                                start=False, stop=(kc == 3),
                                skip_group_check=True,
                            )
                    for mi in range(4, 8):
                        for kc in range(4):
                            nc.tensor.matmul(
                                zp[:, (mi - 4) * 64 + k * 8:(mi - 4) * 64 + k * 8 + 8],
                                whh[:, (kc * 12 + mi) * 128:(kc * 12 + mi + 1) * 128],
                                h_rd[:, kc * 8:(kc + 1) * 8],
                                start=False, stop=(kc == 3),
                                skip_group_check=True,
                            )

                    # TE-only fc work, pre-chain (see fc_mm comment)
                    if j >= 1:
                        if k == 5:
                            fc_mm(j - 1)
                        if k == 6:
                            fc_tr(j - 1)

                    # ---- n-path chain ----
                    rza = work.tile([128, 32], f32, tag="rza")
                    nc.scalar.activation(ap2d(rza, 4, 8), kslice(rp[:], 0, k, 4), Sig)
                    t1 = work.tile([128, 32], f32, tag="t1")
                    nc.vector.tensor_mul(
                        ap2d(t1, 4, 8), ap2d(rza, 4, 8), kslice(nxp[:], 0, k, 4)
                    )
                    t2 = work.tile([128, 32], f32, tag="t2")
                    nc.vector.tensor_add(
                        ap2d(t2, 4, 8), ap2d(t1, 4, 8), kslice(nxp[:], 256, k, 4)
                    )
                    n_t = work.tile([128, 32], f32, tag="n")
                    nc.scalar.activation(n_t[:], t2[:], Tanh)

                    # ---- h update: h' = n + z*(h - n) ----
                    zt = work.tile([128, 32], f32, tag="zt")
                    nc.scalar.activation(ap2d(zt, 4, 8), kslice(zp[:], 0, k, 4), Sig)
                    d_t = work.tile([128, 32], f32, tag="d")
                    nc.vector.tensor_sub(d_t[:], h_rd, n_t[:])
                    v_t = work.tile([128, 32], f32, tag="v")
                    nc.vector.tensor_mul(v_t[:], zt[:], d_t[:])
                    nc.vector.tensor_add(hslot(t + 1), n_t[:], v_t[:])

                    # ---- block-level fillers in the TE-idle tail ----
                    if j + 1 < nblk:
                        if k == 1:
                            r_cur = pgates.tile([128, 256], f32, tag="r")
                            z_cur = pgates.tile([128, 256], f32, tag="z")
                            nx_cur = pgates.tile([128, 512], f32, tag="nx")
                            xp_bias(r_cur, brz8, indr, range(2))
                            xp_bias(z_cur, brz8, indz, range(2))
                        if k == 2:
                            xp_bias(nx_cur, bnx8, indnx, range(4))
                        if k == 3:
                            xp_gemm(j + 1, r_cur, z_cur, nx_cur, range(0, 6))
                        if k == 4:
                            xp_gemm(j + 1, r_cur, z_cur, nx_cur, range(6, 12))
                    if j >= 1:
                        if k == 5:
                            fc_stt(j - 1)
                        if k == 6:
                            fc_copy(j - 1)
                        if k == 7:
                            fc_dma(j - 1)

            fc_mm(nblk - 1)
            fc_stt(nblk - 1)
            fc_tr(nblk - 1)
            fc_copy(nblk - 1)
            fc_dma(nblk - 1)

    nc.compile()
    return nc


def _prep_weights(F_mat, W_ih, W_hh, b_ih, b_hh, fc_W, fc_b):
    bf = ml_dtypes.bfloat16
    WihT = np.ascontiguousarray(W_ih.T).astype(bf)
    WhhT = np.empty((128, 48 * 128), bf)
    for kc in range(4):
        for m in range(12):
            blk = W_hh[m * 128:(m + 1) * 128, kc * 128:(kc + 1) * 128]
            WhhT[:, (kc * 12 + m) * 128:(kc * 12 + m) * 128 + 128] = blk.T.astype(bf)
    brz8 = (b_ih + b_hh)[:2 * HID].reshape(8, 128).astype(bf)
    bnx8 = np.concatenate(
        [b_hh[2 * HID:].reshape(4, 128), b_ih[2 * HID:].reshape(4, 128)], axis=0
    ).astype(bf)
    # tile-major indicator matrices: col = tile*64 + k*8 + b
    cr = np.arange(256)
    indr = np.zeros((8, 256), np.float32)
    indz = np.zeros((8, 256), np.float32)
    for jj in range(4):
        indr[jj] = (cr // 64 == jj)
        indz[4 + jj] = (cr // 64 == jj)
    cn = np.arange(512)
    indnx = np.zeros((8, 512), np.float32)
    for jj in range(4):
        indnx[jj] = (cn < 256) & (cn // 64 == jj)
        indnx[4 + jj] = (cn >= 256) & ((cn - 256) // 64 == jj)
    indr = indr.astype(bf)
    indz = indz.astype(bf)
    indnx = indnx.astype(bf)
    fcWT = np.empty((128, 4 * MST), bf)
    for kc in range(4):
        fcWT[:, kc * MST:(kc + 1) * MST] = fc_W[:, kc * 128:(kc + 1) * 128].T.astype(bf)
    fcb = fc_b.reshape(MST, 1).astype(np.float32)
    return dict(WihT=WihT, WhhT=WhhT, brz8=brz8, bnx8=bnx8,
                indr=indr, indz=indz, indnx=indnx, fcWT=fcWT, fcb=fcb)


def kernel(Y, x0_hat, F_mat, W_ih, W_hh, b_ih, b_hh, fc_W, fc_b):
    from concourse.bass_utils import run_bass_kernel_spmd

    t_steps = Y.shape[1]
    if t_steps not in _compiled:
        _compiled[t_steps] = _build_bass(t_steps)
    nc = _compiled[t_steps]

    w = _prep_weights(F_mat, W_ih, W_hh, b_ih, b_hh, fc_W, fc_b)

    # Host-side prior scan: Xp[b, t] = F^{t+1} x0[b]
    bf = ml_dtypes.bfloat16
    Xp = np.empty((B, t_steps, MST), np.float32)
    X = x0_hat.astype(np.float32)
    FT = F_mat.T.astype(np.float32)
    for t in range(t_steps):
        X = X @ FT
        Xp[:, t] = X

    in_maps = []
    for c in range(NCORES):
        sl = slice(c * BS, (c + 1) * BS)
        # inpT: [128, BS*Tt] bf16, col = b*Tt + t; rows 0:64 = Y^T, 64:128 = Xp^T
        Yc = Y[sl].astype(np.float32)                      # [BS, Tt, 64]
        inpT = np.empty((128, BS * t_steps), bf)
        inpT[0:64] = Yc.transpose(2, 0, 1).reshape(64, -1).astype(bf)
        inpT[64:128] = Xp[sl].transpose(2, 0, 1).reshape(64, -1).astype(bf)
        XpTc = np.ascontiguousarray(
            Xp[sl].transpose(2, 0, 1).reshape(64, -1)
        ).astype(np.float32)
        in_maps.append({"inpT": inpT, "XpT": XpTc, **w})
    trace = os.environ.get("KTRACE") == "1"
    res = run_bass_kernel_spmd(nc, in_maps, list(range(NCORES)), trace=trace)
    global LAST_RESULTS
    LAST_RESULTS = res
    out = np.concatenate([res.results[c]["out"] for c in range(NCORES)], axis=0)
    return out.astype(np.float32)


if __name__ == "__main__":
    rng = np.random.default_rng(0)
    ins = {
        "Y": rng.standard_normal((B, int(os.environ.get("KT", T)), NOBS), dtype=np.float32),
        "x0_hat": rng.standard_normal((B, MST), dtype=np.float32),
        "F_mat": (0.99 * np.linalg.qr(rng.standard_normal((MST, MST)))[0]).astype(np.float32),
        "W_ih": 0.05 * rng.standard_normal((H3, 128), dtype=np.float32),
        "W_hh": 0.05 * rng.standard_normal((H3, HID), dtype=np.float32),
        "b_ih": 0.05 * rng.standard_normal(H3, dtype=np.float32),
        "b_hh": 0.05 * rng.standard_normal(H3, dtype=np.float32),
        "fc_W": 0.05 * rng.standard_normal((MST, HID), dtype=np.float32),
        "fc_b": 0.05 * rng.standard_normal(MST, dtype=np.float32),
    }
    print(kernel(**ins).shape)


# revision 16
# speedup vs baseline: 1.0131x; 1.0131x over previous
"""Trainium2 Bass kernel for nn_MBRNNIncrementEstimator (GRU increment estimator).

Model (per batch b):
  X_prior[t] = F^{t+1} x0                       (linear prior scan)
  x_proj     = concat(Y, X_prior) @ W_ih.T + b_ih
  GRU over T with W_hh, b_hh  -> outs
  out        = X_prior + outs @ fc_W.T + fc_b

Sharding: data-parallel over batch B=64 across 8 cores (8 batches/core).
All on-chip compute uses a transposed layout (features on partitions) so
the GRU per-step vector math uses all 128 lanes.

v3 design:
 - inpT (= [Y; X_prior]^T, bf16) and XpT (f32) are built on the host and
   DMA'd straight in: the prior scan is tiny host math, and doing it there
   removes the whole on-device prologue (~150us).
 - Gate pre-activations live entirely in PSUM: each block's bank is
   initialized by ONE K=8 "indicator" matmul broadcasting the per-tile
   biases bank-wide (start=True zeroes the 2KB zero region), then the
   block-level x-projection GEMM and the per-step recurrent matmuls
   accumulate on top (start=False).
 - Per-step burst order r(16), hn(16), z(16): the r and hn groups retire
   early on the tensor engine's per-instruction semaphore channel
   (~34ns/MM), so the latency-critical n-path (t1 = r*hn -> +xn -> tanh)
   starts ~550ns sooner than with hn last.  The z gate is only needed
   after tanh (h' = n + z*(h-n)), so sig_z runs behind tanh on the
   Scalar engine.
 - Block-level fillers (bias init, x-proj GEMM, fc GEMM/transpose/DMA)
   are emitted AFTER each step's chain ops and spread one small chunk
   per step, so the Tile scheduler cannot interleave them into the
   latency-critical matmul burst (which stretched steps 8.1us -> 3.2us).
"""

import os
import numpy as np
import ml_dtypes

B, T, NOBS, MST, HID = 64, 1024, 64, 64, 512
H3 = 3 * HID
NCORES = 8
BS = B // NCORES            # 8 batches per core
C = 8                       # GRU block length (steps per PSUM block)

_compiled = {}
LAST_RESULTS = None


def _build_bass(t_steps):
    import concourse.bass as bass
    import concourse.mybir as mybir
    import concourse.tile as tile
    from concourse import bacc
    from concourse.masks import make_identity

    f32 = mybir.dt.float32
    bf16 = mybir.dt.bfloat16

    Tt = t_steps
    nblk = Tt // C
    NT = Tt * BS

    nc = bacc.Bacc(None, target_bir_lowering=False)
    inpT_d = nc.declare_dram_parameter("inpT", [128, NT], bf16, isOutput=False)
    XpT_d = nc.declare_dram_parameter("XpT", [MST, NT], f32, isOutput=False)
    WihT_d = nc.declare_dram_parameter("WihT", [128, H3], bf16, isOutput=False)
    WhhT_d = nc.declare_dram_parameter("WhhT", [128, 48 * 128], bf16, isOutput=False)
    brz8_d = nc.declare_dram_parameter("brz8", [128, 128], bf16, isOutput=False)
    bnx8_d = nc.declare_dram_parameter("bnx8", [128, 128], bf16, isOutput=False)
    indr_d = nc.declare_dram_parameter("indr", [128, 256], bf16, isOutput=False)
    indz_d = nc.declare_dram_parameter("indz", [128, 256], bf16, isOutput=False)
    indnx_d = nc.declare_dram_parameter("indnx", [128, 512], bf16, isOutput=False)
    fcWT_d = nc.declare_dram_parameter("fcWT", [128, 4 * MST], bf16, isOutput=False)
    fcb_d = nc.declare_dram_parameter("fcb", [MST, 1], f32, isOutput=False)
    out_d = nc.declare_dram_parameter("out", [BS, Tt, MST], f32, isOutput=True)

    with tile.TileContext(nc) as tc:
        with (
            tc.tile_pool(name="singles", bufs=1) as singles,
            tc.tile_pool(name="work", bufs=4) as work,
            tc.tile_pool(name="pgates", bufs=2, space="PSUM") as pgates,
            tc.tile_pool(name="pfc", bufs=1, space="PSUM") as pfc,
        ):
            # ---- resident tensors ----
            wih = singles.tile([128, H3], bf16)
            whh = singles.tile([128, 48 * 128], bf16)
            fcw = singles.tile([128, 4 * MST], bf16)
            brz8 = singles.tile([128, 128], bf16)
            bnx8 = singles.tile([128, 128], bf16)
            indr = singles.tile([128, 256], bf16)
            indz = singles.tile([128, 256], bf16)
            indnx = singles.tile([128, 512], bf16)
            fcb = singles.tile([MST, 1], f32)
            ident = singles.tile([MST, MST], f32)
            inpT = singles.tile([128, NT], bf16)
            XpT = singles.tile([MST, NT], f32)
            outsT = singles.tile([128, (Tt + 1) * 32], bf16)

            nc.sync.dma_start(wih[:], WihT_d[:])
            nc.sync.dma_start(whh[:], WhhT_d[:])
            nc.sync.dma_start(fcw[:], fcWT_d[:])
            nc.sync.dma_start(brz8[:], brz8_d[:])
            nc.sync.dma_start(bnx8[:], bnx8_d[:])
            nc.sync.dma_start(indr[:], indr_d[:])
            nc.sync.dma_start(indz[:], indz_d[:])
            nc.sync.dma_start(indnx[:], indnx_d[:])
            nc.sync.dma_start(fcb[:], fcb_d[:])
            nc.sync.dma_start(inpT[:], inpT_d[:])
            nc.sync.dma_start(XpT[:], XpT_d[:])
            make_identity(nc, ident[:])
            nc.vector.memset(outsT[:, 0:32], 0.0)  # h_0 = 0

            # PSUM layout per block (three banks, double-buffered):
            #   r bank  [128, 256]: col = mi*64 + k*8 + b   (r gate tiles)
            #   z bank  [128, 256]: col = mi*64 + k*8 + b   (z gate tiles)
            #   nx bank [128, 512]: cols 0:256 = hn, 256:512 = xn
            # Separate r/z/nx tiles give precise per-tile hazards: sig_r
            # gates only on the r matmuls, the z matmuls never WAR-stall
            # against sig_r, and sig_z only sees the z matmuls.

            def kslice(ap_full, base, k, ntile):
                return bass.AP(
                    tensor=ap_full.tensor,
                    offset=ap_full.offset + base + k * 8,
                    ap=[list(ap_full.ap[0]), [64, ntile], [1, 8]],
                )

            def hslot(t):
                return outsT[:, t * 32:(t + 1) * 32]

            def ap2d(t, ncol_outer, stride_outer):
                return bass.AP(
                    tensor=t.tensor, offset=t.offset,
                    ap=[list(t.ap[0]), [stride_outer, ncol_outer], [1, 8]],
                )

            def rhs_inp(jb):
                ia = inpT[:]
                return bass.AP(
                    tensor=ia.tensor, offset=ia.offset + jb * C,
                    ap=[list(ia.ap[0]), [1, C], [Tt, BS]],
                )

            def xp_bias(bp, b8, indt, qs):
                # bank init in FD-128 chunks; operands are zero-padded to
                # K=128 because a K=8 lhsT lowers to a row_grp=q0 sub-tile
                # LDWEIGHTS, which cannot overlap in-flight same-row-group
                # matmuls and serializes the TE queue (~700ns per chunk).
                for q in qs:
                    nc.tensor.matmul(
                        bp[:, q * 128:(q + 1) * 128], b8[:],
                        indt[:, q * 128:(q + 1) * 128],
                        start=(q == 0), stop=False, skip_group_check=True,
                    )

            def xp_gemm(jb, rp, zp, nxp, ms):
                ri = rhs_inp(jb)
                for m in ms:
                    if m < 4:
                        dst = rp[:, m * 64:(m + 1) * 64]
                    elif m < 8:
                        dst = zp[:, (m - 4) * 64:(m - 3) * 64]
                    else:
                        dst = nxp[:, 256 + (m - 8) * 64:256 + (m - 7) * 64]
                    nc.tensor.matmul(
                        dst, wih[:, m * 128:(m + 1) * 128], ri,
                        start=False, stop=(m >= 8), skip_group_check=True,
                    )

            fc_state = {}

            def fc_mm(jb):
                # TE part only — emitted BEFORE the step's chain ops so the
                # per-tile outsT RAW hazard covers only already-written h
                # slots (emitting after the chain made these matmuls wait
                # on that step's h' at the TE queue head, a ~5us/block
                # cascade).
                t0 = jb * C
                oa = outsT[:]
                psf = pfc.tile([MST, 64], f32, tag="fc")
                for kc in range(4):
                    rhs = bass.AP(
                        tensor=oa.tensor,
                        offset=oa.offset + (t0 + 1) * 32 + kc * 8,
                        ap=[list(oa.ap[0]), [1, BS], [32, C]],
                    )  # (b outer, t inner)
                    nc.tensor.matmul(
                        psf[:], fcw[:, kc * MST:(kc + 1) * MST], rhs,
                        start=(kc == 0), stop=(kc == 3),
                    )
                fc_state["psf"] = psf

            def fc_stt(jb):
                t0 = jb * C
                oT = work.tile([MST, 64], f32, tag="oT")
                xa = XpT[:]
                xp_ap = bass.AP(
                    tensor=xa.tensor, offset=xa.offset + t0,
                    ap=[list(xa.ap[0]), [Tt, BS], [1, C]],
                )
                nc.vector.scalar_tensor_tensor(
                    ap2d(oT, BS, 8), fc_state["psf"][:], fcb[:], xp_ap,
                    op0=mybir.AluOpType.add, op1=mybir.AluOpType.add,
                )
                fc_state["oT"] = oT

            def fc_tr(jb):
                ptr = pfc.tile([MST, 64], f32, tag="tr")
                nc.tensor.transpose(ptr[:], fc_state["oT"][:], ident[:])
                fc_state["ptr"] = ptr

            def fc_copy(jb):
                ot = work.tile([MST, 64], f32, tag="ot")
                nc.vector.tensor_copy(ot[:], fc_state["ptr"][:])
                fc_state["ot"] = ot

            def fc_dma(jb):
                t0 = jb * C
                ot = fc_state["ot"]
                for b in range(BS):
                    nc.sync.dma_start(
                        out_d[b, t0:t0 + C, :], ot[b * 8:(b + 1) * 8, :]
                    )

            Sig = mybir.ActivationFunctionType.Sigmoid
            Tanh = mybir.ActivationFunctionType.Tanh

            r_cur = pgates.tile([128, 256], f32, tag="r")
            z_cur = pgates.tile([128, 256], f32, tag="z")
            nx_cur = pgates.tile([128, 512], f32, tag="nx")
            xp_bias(r_cur, brz8, indr, range(2))
            xp_bias(z_cur, brz8, indz, range(2))
            xp_bias(nx_cur, bnx8, indnx, range(4))
            xp_gemm(0, r_cur, z_cur, nx_cur, range(12))

            for j in range(nblk):
                rp, zp, nxp = r_cur, z_cur, nx_cur
                for k in range(C):
                    t = j * C + k
                    h_rd = hslot(t)
                    last_k = (k == C - 1)

                    # burst: r tiles, hn tiles, z tiles.  With separate
                    # r/z PSUM tiles the hazards are precise: sig_r fires
                    # after the 16 r matmuls retire (~550ns into the burst),
                    # t1 after the hn matmuls (~1100ns), sig_z after the z
                    # matmuls (~1650ns, overlapping the tanh path).
                    for mi in range(4):
                        for kc in range(4):
                            nc.tensor.matmul(
                                rp[:, mi * 64 + k * 8:mi * 64 + k * 8 + 8],
                                whh[:, (kc * 12 + mi) * 128:(kc * 12 + mi + 1) * 128],
                                h_rd[:, kc * 8:(kc + 1) * 8],
                                start=False, stop=(kc == 3),
                                skip_group_check=True,
                            )
                    for i in range(4):
                        for kc in range(4):
                            nc.tensor.matmul(
                                nxp[:, i * 64 + k * 8:i * 64 + k * 8 + 8],
                                whh[:, (kc * 12 + 8 + i) * 128:(kc * 12 + 9 + i) * 128],
                                h_rd[:, kc * 8:(kc + 1) * 8],
                                start=False, stop=(kc == 3),
                                skip_group_check=True,
                            )
                    for mi in range(4, 8):
                        for kc in range(4):
                            nc.tensor.matmul(
                                zp[:, (mi - 4) * 64 + k * 8:(mi - 4) * 64 + k * 8 + 8],
                                whh[:, (kc * 12 + mi) * 128:(kc * 12 + mi + 1) * 128],
                                h_rd[:, kc * 8:(kc + 1) * 8],
                                start=False, stop=(kc == 3),
                                skip_group_check=True,
                            )

                    # TE-only fc work, pre-chain (see fc_mm comment)
                    if j >= 1:
                        if k == 5:
                            fc_mm(j - 1)
                        if k == 6:
                            fc_tr(j - 1)

                    # ---- n-path chain ----
                    rza = work.tile([128, 32], f32, tag="rza")
                    nc.scalar.activation(ap2d(rza, 4, 8), kslice(rp[:], 0, k, 4), Sig)
                    t1 = work.tile([128, 32], f32, tag="t1")
                    nc.vector.tensor_mul(
                        ap2d(t1, 4, 8), ap2d(rza, 4, 8), kslice(nxp[:], 0, k, 4)
                    )
                    t2 = work.tile([128, 32], f32, tag="t2")
                    nc.vector.tensor_add(
                        ap2d(t2, 4, 8), ap2d(t1, 4, 8), kslice(nxp[:], 256, k, 4)
                    )
                    n_t = work.tile([128, 32], f32, tag="n")
                    nc.scalar.activation(n_t[:], t2[:], Tanh)

                    # ---- h update: h' = n + z*(h - n) ----
                    zt = work.tile([128, 32], f32, tag="zt")
                    nc.scalar.activation(ap2d(zt, 4, 8), kslice(zp[:], 0, k, 4), Sig)
                    d_t = work.tile([128, 32], f32, tag="d")
                    nc.vector.tensor_sub(d_t[:], h_rd, n_t[:])
                    v_t = work.tile([128, 32], f32, tag="v")
                    nc.vector.tensor_mul(v_t[:], zt[:], d_t[:])
                    nc.vector.tensor_add(hslot(t + 1), n_t[:], v_t[:])

                    # ---- block-level fillers in the TE-idle tail ----
                    if j + 1 < nblk:
                        if k == 1:
                            r_cur = pgates.tile([128, 256], f32, tag="r")
                            z_cur = pgates.tile([128, 256], f32, tag="z")
                            nx_cur = pgates.tile([128, 512], f32, tag="nx")
                            xp_bias(r_cur, brz8, indr, range(2))
                            xp_bias(z_cur, brz8, indz, range(2))
                        if k == 2:
                            xp_bias(nx_cur, bnx8, indnx, range(4))
                        if k == 3:
                            xp_gemm(j + 1, r_cur, z_cur, nx_cur, range(0, 6))
                        if k == 4:
                            xp_gemm(j + 1, r_cur, z_cur, nx_cur, range(6, 12))
                    if j >= 1:
                        if k == 5:
                            fc_stt(j - 1)
                        if k == 6:
                            fc_copy(j - 1)
                        if k == 7:
                            fc_dma(j - 1)

            fc_mm(nblk - 1)
            fc_stt(nblk - 1)
            fc_tr(nblk - 1)
            fc_copy(nblk - 1)
            fc_dma(nblk - 1)

    nc.compile()
    return nc


def _prep_weights(F_mat, W_ih, W_hh, b_ih, b_hh, fc_W, fc_b):
    bf = ml_dtypes.bfloat16
    WihT = np.ascontiguousarray(W_ih.T).astype(bf)
    WhhT = np.empty((128, 48 * 128), bf)
    for kc in range(4):
        for m in range(12):
            blk = W_hh[m * 128:(m + 1) * 128, kc * 128:(kc + 1) * 128]
            WhhT[:, (kc * 12 + m) * 128:(kc * 12 + m) * 128 + 128] = blk.T.astype(bf)
    brz8 = np.zeros((128, 128), np.float32)
    brz8[:8] = (b_ih + b_hh)[:2 * HID].reshape(8, 128)
    brz8 = brz8.astype(bf)
    bnx8 = np.zeros((128, 128), np.float32)
    bnx8[:8] = np.concatenate(
        [b_hh[2 * HID:].reshape(4, 128), b_ih[2 * HID:].reshape(4, 128)], axis=0
    )
    bnx8 = bnx8.astype(bf)
    # tile-major indicator matrices: col = tile*64 + k*8 + b
    cr = np.arange(256)
    indr = np.zeros((128, 256), np.float32)
    indz = np.zeros((128, 256), np.float32)
    for jj in range(4):
        indr[jj] = (cr // 64 == jj)
        indz[4 + jj] = (cr // 64 == jj)
    cn = np.arange(512)
    indnx = np.zeros((128, 512), np.float32)
    for jj in range(4):
        indnx[jj] = (cn < 256) & (cn // 64 == jj)
        indnx[4 + jj] = (cn >= 256) & ((cn - 256) // 64 == jj)
    indr = indr.astype(bf)
    indz = indz.astype(bf)
    indnx = indnx.astype(bf)
    fcWT = np.empty((128, 4 * MST), bf)
    for kc in range(4):
        fcWT[:, kc * MST:(kc + 1) * MST] = fc_W[:, kc * 128:(kc + 1) * 128].T.astype(bf)
    fcb = fc_b.reshape(MST, 1).astype(np.float32)
    return dict(WihT=WihT, WhhT=WhhT, brz8=brz8, bnx8=bnx8,
                indr=indr, indz=indz, indnx=indnx, fcWT=fcWT, fcb=fcb)


def kernel(Y, x0_hat, F_mat, W_ih, W_hh, b_ih, b_hh, fc_W, fc_b):
    from concourse.bass_utils import run_bass_kernel_spmd

    t_steps = Y.shape[1]
    if t_steps not in _compiled:
        _compiled[t_steps] = _build_bass(t_steps)
    nc = _compiled[t_steps]

    w = _prep_weights(F_mat, W_ih, W_hh, b_ih, b_hh, fc_W, fc_b)

    # Host-side prior scan: Xp[b, t] = F^{t+1} x0[b]
    bf = ml_dtypes.bfloat16
    Xp = np.empty((B, t_steps, MST), np.float32)
    X = x0_hat.astype(np.float32)
    FT = F_mat.T.astype(np.float32)
    for t in range(t_steps):
        X = X @ FT
        Xp[:, t] = X

    in_maps = []
    for c in range(NCORES):
        sl = slice(c * BS, (c + 1) * BS)
        # inpT: [128, BS*Tt] bf16, col = b*Tt + t; rows 0:64 = Y^T, 64:128 = Xp^T
        Yc = Y[sl].astype(np.float32)                      # [BS, Tt, 64]
        inpT = np.empty((128, BS * t_steps), bf)
        inpT[0:64] = Yc.transpose(2, 0, 1).reshape(64, -1).astype(bf)
        inpT[64:128] = Xp[sl].transpose(2, 0, 1).reshape(64, -1).astype(bf)
        XpTc = np.ascontiguousarray(
            Xp[sl].transpose(2, 0, 1).reshape(64, -1)
        ).astype(np.float32)
        in_maps.append({"inpT": inpT, "XpT": XpTc, **w})
    trace = os.environ.get("KTRACE") == "1"
    res = run_bass_kernel_spmd(nc, in_maps, list(range(NCORES)), trace=trace)
    global LAST_RESULTS
    LAST_RESULTS = res
    out = np.concatenate([res.results[c]["out"] for c in range(NCORES)], axis=0)
    return out.astype(np.float32)


if __name__ == "__main__":
    rng = np.random.default_rng(0)
    ins = {
        "Y": rng.standard_normal((B, int(os.environ.get("KT", T)), NOBS), dtype=np.float32),
        "x0_hat": rng.standard_normal((B, MST), dtype=np.float32),
        "F_mat": (0.99 * np.linalg.qr(rng.standard_normal((MST, MST)))[0]).astype(np.float32),
        "W_ih": 0.05 * rng.standard_normal((H3, 128), dtype=np.float32),
        "W_hh": 0.05 * rng.standard_normal((H3, HID), dtype=np.float32),
        "b_ih": 0.05 * rng.standard_normal(H3, dtype=np.float32),
        "b_hh": 0.05 * rng.standard_normal(H3, dtype=np.float32),
        "fc_W": 0.05 * rng.standard_normal((MST, HID), dtype=np.float32),
        "fc_b": 0.05 * rng.standard_normal(MST, dtype=np.float32),
    }
    print(kernel(**ins).shape)


# revision 17
# speedup vs baseline: 1.0132x; 1.0001x over previous
"""Trainium2 Bass kernel for nn_MBRNNIncrementEstimator (GRU increment estimator).

Model (per batch b):
  X_prior[t] = F^{t+1} x0                       (linear prior scan)
  x_proj     = concat(Y, X_prior) @ W_ih.T + b_ih
  GRU over T with W_hh, b_hh  -> outs
  out        = X_prior + outs @ fc_W.T + fc_b

Sharding: data-parallel over batch B=64 across 8 cores (8 batches/core).
All on-chip compute uses a transposed layout (features on partitions) so
the GRU per-step vector math uses all 128 lanes.

v3 design:
 - inpT (= [Y; X_prior]^T, bf16) and XpT (f32) are built on the host and
   DMA'd straight in: the prior scan is tiny host math, and doing it there
   removes the whole on-device prologue (~150us).
 - Gate pre-activations live entirely in PSUM: each block's bank is
   initialized by ONE K=8 "indicator" matmul broadcasting the per-tile
   biases bank-wide (start=True zeroes the 2KB zero region), then the
   block-level x-projection GEMM and the per-step recurrent matmuls
   accumulate on top (start=False).
 - Per-step burst order r(16), hn(16), z(16): the r and hn groups retire
   early on the tensor engine's per-instruction semaphore channel
   (~34ns/MM), so the latency-critical n-path (t1 = r*hn -> +xn -> tanh)
   starts ~550ns sooner than with hn last.  The z gate is only needed
   after tanh (h' = n + z*(h-n)), so sig_z runs behind tanh on the
   Scalar engine.
 - Block-level fillers (bias init, x-proj GEMM, fc GEMM/transpose/DMA)
   are emitted AFTER each step's chain ops and spread one small chunk
   per step, so the Tile scheduler cannot interleave them into the
   latency-critical matmul burst (which stretched steps 8.1us -> 3.2us).
"""

import os
import numpy as np
import ml_dtypes

B, T, NOBS, MST, HID = 64, 1024, 64, 64, 512
H3 = 3 * HID
NCORES = 8
BS = B // NCORES            # 8 batches per core
C = 8                       # GRU block length (steps per PSUM block)

_compiled = {}
LAST_RESULTS = None


def _build_bass(t_steps):
    import concourse.bass as bass
    import concourse.mybir as mybir
    import concourse.tile as tile
    from concourse import bacc
    from concourse.masks import make_identity

    f32 = mybir.dt.float32
    bf16 = mybir.dt.bfloat16

    Tt = t_steps
    nblk = Tt // C
    NT = Tt * BS

    nc = bacc.Bacc(None, target_bir_lowering=False)
    inpT_d = nc.declare_dram_parameter("inpT", [128, NT], bf16, isOutput=False)
    XpT_d = nc.declare_dram_parameter("XpT", [MST, NT], f32, isOutput=False)
    WihT_d = nc.declare_dram_parameter("WihT", [128, H3], bf16, isOutput=False)
    WhhT_d = nc.declare_dram_parameter("WhhT", [128, 48 * 128], bf16, isOutput=False)
    brz8_d = nc.declare_dram_parameter("brz8", [128, 128], bf16, isOutput=False)
    bnx8_d = nc.declare_dram_parameter("bnx8", [128, 128], bf16, isOutput=False)
    indr_d = nc.declare_dram_parameter("indr", [128, 256], bf16, isOutput=False)
    indz_d = nc.declare_dram_parameter("indz", [128, 256], bf16, isOutput=False)
    indnx_d = nc.declare_dram_parameter("indnx", [128, 512], bf16, isOutput=False)
    fcWT_d = nc.declare_dram_parameter("fcWT", [128, 4 * MST], bf16, isOutput=False)
    fcb_d = nc.declare_dram_parameter("fcb", [MST, 1], f32, isOutput=False)
    out_d = nc.declare_dram_parameter("out", [BS, Tt, MST], f32, isOutput=True)

    with tile.TileContext(nc) as tc:
        with (
            tc.tile_pool(name="singles", bufs=1) as singles,
            tc.tile_pool(name="work", bufs=4) as work,
            tc.tile_pool(name="pgates", bufs=2, space="PSUM") as pgates,
            tc.tile_pool(name="pfc", bufs=1, space="PSUM") as pfc,
        ):
            # ---- resident tensors ----
            wih = singles.tile([128, H3], bf16)
            whh = singles.tile([128, 48 * 128], bf16)
            fcw = singles.tile([128, 4 * MST], bf16)
            brz8 = singles.tile([128, 128], bf16)
            bnx8 = singles.tile([128, 128], bf16)
            indr = singles.tile([128, 256], bf16)
            indz = singles.tile([128, 256], bf16)
            indnx = singles.tile([128, 512], bf16)
            fcb = singles.tile([MST, 1], f32)
            ident = singles.tile([MST, MST], f32)
            inpT = singles.tile([128, NT], bf16)
            XpT = singles.tile([MST, NT], f32)
            outsT = singles.tile([128, (Tt + 1) * 32], bf16)

            nc.sync.dma_start(wih[:], WihT_d[:])
            nc.sync.dma_start(whh[:], WhhT_d[:])
            nc.sync.dma_start(fcw[:], fcWT_d[:])
            nc.sync.dma_start(brz8[:], brz8_d[:])
            nc.sync.dma_start(bnx8[:], bnx8_d[:])
            nc.sync.dma_start(indr[:], indr_d[:])
            nc.sync.dma_start(indz[:], indz_d[:])
            nc.sync.dma_start(indnx[:], indnx_d[:])
            nc.sync.dma_start(fcb[:], fcb_d[:])
            nc.sync.dma_start(inpT[:], inpT_d[:])
            nc.sync.dma_start(XpT[:], XpT_d[:])
            make_identity(nc, ident[:])
            nc.vector.memset(outsT[:, 0:32], 0.0)  # h_0 = 0

            # PSUM layout per block (three banks, double-buffered):
            #   r bank  [128, 256]: col = mi*64 + k*8 + b   (r gate tiles)
            #   z bank  [128, 256]: col = mi*64 + k*8 + b   (z gate tiles)
            #   nx bank [128, 512]: cols 0:256 = hn, 256:512 = xn
            # Separate r/z/nx tiles give precise per-tile hazards: sig_r
            # gates only on the r matmuls, the z matmuls never WAR-stall
            # against sig_r, and sig_z only sees the z matmuls.

            def kslice(ap_full, base, k, ntile):
                return bass.AP(
                    tensor=ap_full.tensor,
                    offset=ap_full.offset + base + k * 8,
                    ap=[list(ap_full.ap[0]), [64, ntile], [1, 8]],
                )

            def hslot(t):
                return outsT[:, t * 32:(t + 1) * 32]

            def ap2d(t, ncol_outer, stride_outer):
                return bass.AP(
                    tensor=t.tensor, offset=t.offset,
                    ap=[list(t.ap[0]), [stride_outer, ncol_outer], [1, 8]],
                )

            def rhs_inp(jb):
                ia = inpT[:]
                return bass.AP(
                    tensor=ia.tensor, offset=ia.offset + jb * C,
                    ap=[list(ia.ap[0]), [1, C], [Tt, BS]],
                )

            def xp_bias(bp, b8, indt, qs):
                # bank init in FD-128 chunks; operands are zero-padded to
                # K=128 because a K=8 lhsT lowers to a row_grp=q0 sub-tile
                # LDWEIGHTS, which cannot overlap in-flight same-row-group
                # matmuls and serializes the TE queue (~700ns per chunk).
                for q in qs:
                    nc.tensor.matmul(
                        bp[:, q * 128:(q + 1) * 128], b8[:],
                        indt[:, q * 128:(q + 1) * 128],
                        start=(q == 0), stop=False, skip_group_check=True,
                    )

            def xp_gemm(jb, rp, zp, nxp, ms):
                ri = rhs_inp(jb)
                for m in ms:
                    if m < 4:
                        dst = rp[:, m * 64:(m + 1) * 64]
                    elif m < 8:
                        dst = zp[:, (m - 4) * 64:(m - 3) * 64]
                    else:
                        dst = nxp[:, 256 + (m - 8) * 64:256 + (m - 7) * 64]
                    nc.tensor.matmul(
                        dst, wih[:, m * 128:(m + 1) * 128], ri,
                        start=False, stop=(m >= 8), skip_group_check=True,
                    )

            fc_state = {}

            def fc_mm(jb):
                # TE part only — emitted BEFORE the step's chain ops so the
                # per-tile outsT RAW hazard covers only already-written h
                # slots (emitting after the chain made these matmuls wait
                # on that step's h' at the TE queue head, a ~5us/block
                # cascade).
                t0 = jb * C
                oa = outsT[:]
                psf = pfc.tile([MST, 64], f32, tag="fc")
                for kc in range(4):
                    rhs = bass.AP(
                        tensor=oa.tensor,
                        offset=oa.offset + (t0 + 1) * 32 + kc * 8,
                        ap=[list(oa.ap[0]), [1, BS], [32, C]],
                    )  # (b outer, t inner)
                    nc.tensor.matmul(
                        psf[:], fcw[:, kc * MST:(kc + 1) * MST], rhs,
                        start=(kc == 0), stop=(kc == 3),
                    )
                fc_state["psf"] = psf

            def fc_stt(jb):
                t0 = jb * C
                oT = work.tile([MST, 64], f32, tag="oT")
                xa = XpT[:]
                xp_ap = bass.AP(
                    tensor=xa.tensor, offset=xa.offset + t0,
                    ap=[list(xa.ap[0]), [Tt, BS], [1, C]],
                )
                nc.vector.scalar_tensor_tensor(
                    ap2d(oT, BS, 8), fc_state["psf"][:], fcb[:], xp_ap,
                    op0=mybir.AluOpType.add, op1=mybir.AluOpType.add,
                )
                fc_state["oT"] = oT

            def fc_tr(jb):
                ptr = pfc.tile([MST, 64], f32, tag="tr")
                nc.tensor.transpose(ptr[:], fc_state["oT"][:], ident[:])
                fc_state["ptr"] = ptr

            def fc_copy(jb):
                ot = work.tile([MST, 64], f32, tag="ot")
                nc.vector.tensor_copy(ot[:], fc_state["ptr"][:])
                fc_state["ot"] = ot

            def fc_dma(jb):
                t0 = jb * C
                ot = fc_state["ot"]
                for b in range(BS):
                    nc.sync.dma_start(
                        out_d[b, t0:t0 + C, :], ot[b * 8:(b + 1) * 8, :]
                    )

            Sig = mybir.ActivationFunctionType.Sigmoid
            Tanh = mybir.ActivationFunctionType.Tanh

            r_cur = pgates.tile([128, 256], f32, tag="r")
            z_cur = pgates.tile([128, 256], f32, tag="z")
            nx_cur = pgates.tile([128, 512], f32, tag="nx")
            xp_bias(r_cur, brz8, indr, range(2))
            xp_bias(z_cur, brz8, indz, range(2))
            xp_bias(nx_cur, bnx8, indnx, range(4))
            xp_gemm(0, r_cur, z_cur, nx_cur, range(12))

            for j in range(nblk):
                rp, zp, nxp = r_cur, z_cur, nx_cur
                for k in range(C):
                    t = j * C + k
                    h_rd = hslot(t)
                    last_k = (k == C - 1)

                    # burst: r tiles, hn tiles, z tiles.  With separate
                    # r/z PSUM tiles the hazards are precise: sig_r fires
                    # after the 16 r matmuls retire (~550ns into the burst),
                    # t1 after the hn matmuls (~1100ns), sig_z after the z
                    # matmuls (~1650ns, overlapping the tanh path).
                    for mi in range(4):
                        for kc in range(4):
                            nc.tensor.matmul(
                                rp[:, mi * 64 + k * 8:mi * 64 + k * 8 + 8],
                                whh[:, (kc * 12 + mi) * 128:(kc * 12 + mi + 1) * 128],
                                h_rd[:, kc * 8:(kc + 1) * 8],
                                start=False, stop=(kc == 3),
                                skip_group_check=True,
                            )
                    for i in range(4):
                        for kc in range(4):
                            nc.tensor.matmul(
                                nxp[:, i * 64 + k * 8:i * 64 + k * 8 + 8],
                                whh[:, (kc * 12 + 8 + i) * 128:(kc * 12 + 9 + i) * 128],
                                h_rd[:, kc * 8:(kc + 1) * 8],
                                start=False, stop=(kc == 3),
                                skip_group_check=True,
                            )
                    for mi in range(4, 8):
                        for kc in range(4):
                            nc.tensor.matmul(
                                zp[:, (mi - 4) * 64 + k * 8:(mi - 4) * 64 + k * 8 + 8],
                                whh[:, (kc * 12 + mi) * 128:(kc * 12 + mi + 1) * 128],
                                h_rd[:, kc * 8:(kc + 1) * 8],
                                start=False, stop=(kc == 3),
                                skip_group_check=True,
                            )

                    # TE-only fc work, pre-chain (see fc_mm comment)
                    if j >= 1:
                        if k == 5:
                            fc_mm(j - 1)
                        if k == 6:
                            fc_tr(j - 1)

                    # ---- n-path chain ----
                    rza = work.tile([128, 32], f32, tag="rza")
                    nc.scalar.activation(ap2d(rza, 4, 8), kslice(rp[:], 0, k, 4), Sig)
                    t1 = work.tile([128, 32], f32, tag="t1")
                    nc.vector.tensor_mul(
                        ap2d(t1, 4, 8), ap2d(rza, 4, 8), kslice(nxp[:], 0, k, 4)
                    )
                    t2 = work.tile([128, 32], f32, tag="t2")
                    nc.vector.tensor_add(
                        ap2d(t2, 4, 8), ap2d(t1, 4, 8), kslice(nxp[:], 256, k, 4)
                    )
                    n_t = work.tile([128, 32], f32, tag="n")
                    nc.scalar.activation(n_t[:], t2[:], Tanh)

                    # ---- h update: h' = n + z*(h - n) ----
                    zt = work.tile([128, 32], f32, tag="zt")
                    nc.scalar.activation(ap2d(zt, 4, 8), kslice(zp[:], 0, k, 4), Sig)
                    d_t = work.tile([128, 32], f32, tag="d")
                    nc.vector.tensor_sub(d_t[:], h_rd, n_t[:])
                    v_t = work.tile([128, 32], f32, tag="v")
                    nc.vector.tensor_mul(v_t[:], zt[:], d_t[:])
                    nc.vector.tensor_add(hslot(t + 1), n_t[:], v_t[:])

                    # ---- block-level fillers in the TE-idle tail ----
                    # Emitted late in the block (k4-k6) so the WAR edges on
                    # the next block's buffers (last read by block j-1's
                    # final-step chain) are several steps stale by the time
                    # the tensor engine reaches these at the queue head.
                    if j + 1 < nblk:
                        if k == 4:
                            r_cur = pgates.tile([128, 256], f32, tag="r")
                            z_cur = pgates.tile([128, 256], f32, tag="z")
                            nx_cur = pgates.tile([128, 512], f32, tag="nx")
                            xp_bias(r_cur, brz8, indr, range(2))
                            xp_bias(z_cur, brz8, indz, range(2))
                            xp_bias(nx_cur, bnx8, indnx, range(4))
                        if k == 5:
                            xp_gemm(j + 1, r_cur, z_cur, nx_cur, range(0, 6))
                        if k == 6:
                            xp_gemm(j + 1, r_cur, z_cur, nx_cur, range(6, 12))
                    if j >= 1:
                        if k == 5:
                            fc_stt(j - 1)
                        if k == 6:
                            fc_copy(j - 1)
                        if k == 7:
                            fc_dma(j - 1)

            fc_mm(nblk - 1)
            fc_stt(nblk - 1)
            fc_tr(nblk - 1)
            fc_copy(nblk - 1)
            fc_dma(nblk - 1)

    nc.compile()
    return nc


def _prep_weights(F_mat, W_ih, W_hh, b_ih, b_hh, fc_W, fc_b):
    bf = ml_dtypes.bfloat16
    WihT = np.ascontiguousarray(W_ih.T).astype(bf)
    WhhT = np.empty((128, 48 * 128), bf)
    for kc in range(4):
        for m in range(12):
            blk = W_hh[m * 128:(m + 1) * 128, kc * 128:(kc + 1) * 128]
            WhhT[:, (kc * 12 + m) * 128:(kc * 12 + m) * 128 + 128] = blk.T.astype(bf)
    brz8 = np.zeros((128, 128), np.float32)
    brz8[:8] = (b_ih + b_hh)[:2 * HID].reshape(8, 128)
    brz8 = brz8.astype(bf)
    bnx8 = np.zeros((128, 128), np.float32)
    bnx8[:8] = np.concatenate(
        [b_hh[2 * HID:].reshape(4, 128), b_ih[2 * HID:].reshape(4, 128)], axis=0
    )
    bnx8 = bnx8.astype(bf)
    # tile-major indicator matrices: col = tile*64 + k*8 + b
    cr = np.arange(256)
    indr = np.zeros((128, 256), np.float32)
    indz = np.zeros((128, 256), np.float32)
    for jj in range(4):
        indr[jj] = (cr // 64 == jj)
        indz[4 + jj] = (cr // 64 == jj)
    cn = np.arange(512)
    indnx = np.zeros((128, 512), np.float32)
    for jj in range(4):
        indnx[jj] = (cn < 256) & (cn // 64 == jj)
        indnx[4 + jj] = (cn >= 256) & ((cn - 256) // 64 == jj)
    indr = indr.astype(bf)
    indz = indz.astype(bf)
    indnx = indnx.astype(bf)
    fcWT = np.empty((128, 4 * MST), bf)
    for kc in range(4):
        fcWT[:, kc * MST:(kc + 1) * MST] = fc_W[:, kc * 128:(kc + 1) * 128].T.astype(bf)
    fcb = fc_b.reshape(MST, 1).astype(np.float32)
    return dict(WihT=WihT, WhhT=WhhT, brz8=brz8, bnx8=bnx8,
                indr=indr, indz=indz, indnx=indnx, fcWT=fcWT, fcb=fcb)


def kernel(Y, x0_hat, F_mat, W_ih, W_hh, b_ih, b_hh, fc_W, fc_b):
    from concourse.bass_utils import run_bass_kernel_spmd

    t_steps = Y.shape[1]
    if t_steps not in _compiled:
        _compiled[t_steps] = _build_bass(t_steps)
    nc = _compiled[t_steps]

    w = _prep_weights(F_mat, W_ih, W_hh, b_ih, b_hh, fc_W, fc_b)

    # Host-side prior scan: Xp[b, t] = F^{t+1} x0[b]
    bf = ml_dtypes.bfloat16
    Xp = np.empty((B, t_steps, MST), np.float32)
    X = x0_hat.astype(np.float32)
    FT = F_mat.T.astype(np.float32)
    for t in range(t_steps):
        X = X @ FT
        Xp[:, t] = X

    in_maps = []
    for c in range(NCORES):
        sl = slice(c * BS, (c + 1) * BS)
        # inpT: [128, BS*Tt] bf16, col = b*Tt + t; rows 0:64 = Y^T, 64:128 = Xp^T
        Yc = Y[sl].astype(np.float32)                      # [BS, Tt, 64]
        inpT = np.empty((128, BS * t_steps), bf)
        inpT[0:64] = Yc.transpose(2, 0, 1).reshape(64, -1).astype(bf)
        inpT[64:128] = Xp[sl].transpose(2, 0, 1).reshape(64, -1).astype(bf)
        XpTc = np.ascontiguousarray(
            Xp[sl].transpose(2, 0, 1).reshape(64, -1)
        ).astype(np.float32)
        in_maps.append({"inpT": inpT, "XpT": XpTc, **w})
    trace = os.environ.get("KTRACE") == "1"
    res = run_bass_kernel_spmd(nc, in_maps, list(range(NCORES)), trace=trace)
    global LAST_RESULTS
    LAST_RESULTS = res
    out = np.concatenate([res.results[c]["out"] for c in range(NCORES)], axis=0)
    return out.astype(np.float32)


if __name__ == "__main__":
    rng = np.random.default_rng(0)
    ins = {
        "Y": rng.standard_normal((B, int(os.environ.get("KT", T)), NOBS), dtype=np.float32),
        "x0_hat": rng.standard_normal((B, MST), dtype=np.float32),
        "F_mat": (0.99 * np.linalg.qr(rng.standard_normal((MST, MST)))[0]).astype(np.float32),
        "W_ih": 0.05 * rng.standard_normal((H3, 128), dtype=np.float32),
        "W_hh": 0.05 * rng.standard_normal((H3, HID), dtype=np.float32),
        "b_ih": 0.05 * rng.standard_normal(H3, dtype=np.float32),
        "b_hh": 0.05 * rng.standard_normal(H3, dtype=np.float32),
        "fc_W": 0.05 * rng.standard_normal((MST, HID), dtype=np.float32),
        "fc_b": 0.05 * rng.standard_normal(MST, dtype=np.float32),
    }
    print(kernel(**ins).shape)


# revision 18
# speedup vs baseline: 1.0135x; 1.0003x over previous
"""Trainium2 Bass kernel for nn_MBRNNIncrementEstimator (GRU increment estimator).

Model (per batch b):
  X_prior[t] = F^{t+1} x0                       (linear prior scan)
  x_proj     = concat(Y, X_prior) @ W_ih.T + b_ih
  GRU over T with W_hh, b_hh  -> outs
  out        = X_prior + outs @ fc_W.T + fc_b

Sharding: data-parallel over batch B=64 across 8 cores (8 batches/core).
All on-chip compute uses a transposed layout (features on partitions) so
the GRU per-step vector math uses all 128 lanes.

v3 design:
 - inpT (= [Y; X_prior]^T, bf16) and XpT (f32) are built on the host and
   DMA'd straight in: the prior scan is tiny host math, and doing it there
   removes the whole on-device prologue (~150us).
 - Gate pre-activations live entirely in PSUM: each block's bank is
   initialized by ONE K=8 "indicator" matmul broadcasting the per-tile
   biases bank-wide (start=True zeroes the 2KB zero region), then the
   block-level x-projection GEMM and the per-step recurrent matmuls
   accumulate on top (start=False).
 - Per-step burst order r(16), hn(16), z(16): the r and hn groups retire
   early on the tensor engine's per-instruction semaphore channel
   (~34ns/MM), so the latency-critical n-path (t1 = r*hn -> +xn -> tanh)
   starts ~550ns sooner than with hn last.  The z gate is only needed
   after tanh (h' = n + z*(h-n)), so sig_z runs behind tanh on the
   Scalar engine.
 - Block-level fillers (bias init, x-proj GEMM, fc GEMM/transpose/DMA)
   are emitted AFTER each step's chain ops and spread one small chunk
   per step, so the Tile scheduler cannot interleave them into the
   latency-critical matmul burst (which stretched steps 8.1us -> 3.2us).
"""

import os
import numpy as np
import ml_dtypes

B, T, NOBS, MST, HID = 64, 1024, 64, 64, 512
H3 = 3 * HID
NCORES = 8
BS = B // NCORES            # 8 batches per core
C = 8                       # GRU block length (steps per PSUM block)

_compiled = {}
LAST_RESULTS = None


def _build_bass(t_steps):
    import concourse.bass as bass
    import concourse.mybir as mybir
    import concourse.tile as tile
    from concourse import bacc
    from concourse.masks import make_identity

    f32 = mybir.dt.float32
    bf16 = mybir.dt.bfloat16

    Tt = t_steps
    nblk = Tt // C
    NT = Tt * BS

    nc = bacc.Bacc(None, target_bir_lowering=False)
    inpT_d = nc.declare_dram_parameter("inpT", [128, NT], bf16, isOutput=False)
    XpT_d = nc.declare_dram_parameter("XpT", [MST, NT], f32, isOutput=False)
    WihT_d = nc.declare_dram_parameter("WihT", [128, H3], bf16, isOutput=False)
    WhhT_d = nc.declare_dram_parameter("WhhT", [128, 48 * 128], bf16, isOutput=False)
    brz8_d = nc.declare_dram_parameter("brz8", [128, 128], bf16, isOutput=False)
    bnx8_d = nc.declare_dram_parameter("bnx8", [128, 128], bf16, isOutput=False)
    indr_d = nc.declare_dram_parameter("indr", [128, 256], bf16, isOutput=False)
    indz_d = nc.declare_dram_parameter("indz", [128, 256], bf16, isOutput=False)
    indnx_d = nc.declare_dram_parameter("indnx", [128, 512], bf16, isOutput=False)
    fcWT_d = nc.declare_dram_parameter("fcWT", [128, 4 * MST], bf16, isOutput=False)
    fcb_d = nc.declare_dram_parameter("fcb", [MST, 1], f32, isOutput=False)
    out_d = nc.declare_dram_parameter("out", [BS, Tt, MST], f32, isOutput=True)

    with tile.TileContext(nc) as tc:
        with (
            tc.tile_pool(name="singles", bufs=1) as singles,
            tc.tile_pool(name="work", bufs=4) as work,
            tc.tile_pool(name="pgates", bufs=2, space="PSUM") as pgates,
            tc.tile_pool(name="pfc", bufs=1, space="PSUM") as pfc,
        ):
            # ---- resident tensors ----
            wih = singles.tile([128, H3], bf16)
            whh = singles.tile([128, 48 * 128], bf16)
            fcw = singles.tile([128, 4 * MST], bf16)
            brz8 = singles.tile([128, 128], bf16)
            bnx8 = singles.tile([128, 128], bf16)
            indr = singles.tile([128, 256], bf16)
            indz = singles.tile([128, 256], bf16)
            indnx = singles.tile([128, 512], bf16)
            fcb = singles.tile([MST, 1], f32)
            ident = singles.tile([MST, MST], f32)
            inpT = singles.tile([128, NT], bf16)
            XpT = singles.tile([MST, NT], f32)
            outsT = singles.tile([128, (Tt + 1) * 32], bf16)

            nc.sync.dma_start(wih[:], WihT_d[:])
            nc.sync.dma_start(whh[:], WhhT_d[:])
            nc.sync.dma_start(fcw[:], fcWT_d[:])
            nc.sync.dma_start(brz8[:], brz8_d[:])
            nc.sync.dma_start(bnx8[:], bnx8_d[:])
            nc.sync.dma_start(indr[:], indr_d[:])
            nc.sync.dma_start(indz[:], indz_d[:])
            nc.sync.dma_start(indnx[:], indnx_d[:])
            nc.sync.dma_start(fcb[:], fcb_d[:])
            nc.sync.dma_start(inpT[:], inpT_d[:])
            nc.sync.dma_start(XpT[:], XpT_d[:])
            make_identity(nc, ident[:])
            nc.vector.memset(outsT[:, 0:32], 0.0)  # h_0 = 0

            # PSUM layout per block (three banks, double-buffered):
            #   r bank  [128, 256]: col = mi*64 + k*8 + b   (r gate tiles)
            #   z bank  [128, 256]: col = mi*64 + k*8 + b   (z gate tiles)
            #   nx bank [128, 512]: cols 0:256 = hn, 256:512 = xn
            # Separate r/z/nx tiles give precise per-tile hazards: sig_r
            # gates only on the r matmuls, the z matmuls never WAR-stall
            # against sig_r, and sig_z only sees the z matmuls.

            def kslice(ap_full, base, k, ntile):
                return bass.AP(
                    tensor=ap_full.tensor,
                    offset=ap_full.offset + base + k * 8,
                    ap=[list(ap_full.ap[0]), [64, ntile], [1, 8]],
                )

            def hslot(t):
                return outsT[:, t * 32:(t + 1) * 32]

            def ap2d(t, ncol_outer, stride_outer):
                return bass.AP(
                    tensor=t.tensor, offset=t.offset,
                    ap=[list(t.ap[0]), [stride_outer, ncol_outer], [1, 8]],
                )

            def rhs_inp(jb):
                ia = inpT[:]
                return bass.AP(
                    tensor=ia.tensor, offset=ia.offset + jb * C,
                    ap=[list(ia.ap[0]), [1, C], [Tt, BS]],
                )

            def xp_bias(bp, b8, indt, qs):
                # bank init in FD-128 chunks; operands are zero-padded to
                # K=128 because a K=8 lhsT lowers to a row_grp=q0 sub-tile
                # LDWEIGHTS, which cannot overlap in-flight same-row-group
                # matmuls and serializes the TE queue (~700ns per chunk).
                for q in qs:
                    nc.tensor.matmul(
                        bp[:, q * 128:(q + 1) * 128], b8[:],
                        indt[:, q * 128:(q + 1) * 128],
                        start=(q == 0), stop=False, skip_group_check=True,
                    )

            def xp_gemm(jb, rp, zp, nxp, ms):
                ri = rhs_inp(jb)
                for m in ms:
                    if m < 4:
                        dst = rp[:, m * 64:(m + 1) * 64]
                    elif m < 8:
                        dst = zp[:, (m - 4) * 64:(m - 3) * 64]
                    else:
                        dst = nxp[:, 256 + (m - 8) * 64:256 + (m - 7) * 64]
                    nc.tensor.matmul(
                        dst, wih[:, m * 128:(m + 1) * 128], ri,
                        start=False, stop=(m >= 8), skip_group_check=True,
                    )

            fc_state = {}

            def fc_mm(jb):
                # TE part only — emitted BEFORE the step's chain ops so the
                # per-tile outsT RAW hazard covers only already-written h
                # slots (emitting after the chain made these matmuls wait
                # on that step's h' at the TE queue head, a ~5us/block
                # cascade).
                t0 = jb * C
                oa = outsT[:]
                psf = pfc.tile([MST, 64], f32, tag="fc")
                for kc in range(4):
                    rhs = bass.AP(
                        tensor=oa.tensor,
                        offset=oa.offset + (t0 + 1) * 32 + kc * 8,
                        ap=[list(oa.ap[0]), [1, BS], [32, C]],
                    )  # (b outer, t inner)
                    nc.tensor.matmul(
                        psf[:], fcw[:, kc * MST:(kc + 1) * MST], rhs,
                        start=(kc == 0), stop=(kc == 3),
                    )
                fc_state["psf"] = psf

            def fc_stt(jb):
                t0 = jb * C
                oT = work.tile([MST, 64], f32, tag="oT")
                xa = XpT[:]
                xp_ap = bass.AP(
                    tensor=xa.tensor, offset=xa.offset + t0,
                    ap=[list(xa.ap[0]), [Tt, BS], [1, C]],
                )
                nc.vector.scalar_tensor_tensor(
                    ap2d(oT, BS, 8), fc_state["psf"][:], fcb[:], xp_ap,
                    op0=mybir.AluOpType.add, op1=mybir.AluOpType.add,
                )
                fc_state["oT"] = oT

            def fc_tr(jb):
                ptr = pfc.tile([MST, 64], f32, tag="tr")
                nc.tensor.transpose(ptr[:], fc_state["oT"][:], ident[:])
                fc_state["ptr"] = ptr

            def fc_copy(jb):
                ot = work.tile([MST, 64], f32, tag="ot")
                nc.vector.tensor_copy(ot[:], fc_state["ptr"][:])
                fc_state["ot"] = ot

            def fc_dma(jb):
                t0 = jb * C
                ot = fc_state["ot"]
                for b in range(BS):
                    nc.sync.dma_start(
                        out_d[b, t0:t0 + C, :], ot[b * 8:(b + 1) * 8, :]
                    )

            Sig = mybir.ActivationFunctionType.Sigmoid
            Tanh = mybir.ActivationFunctionType.Tanh

            r_cur = pgates.tile([128, 256], f32, tag="r")
            z_cur = pgates.tile([128, 256], f32, tag="z")
            nx_cur = pgates.tile([128, 512], f32, tag="nx")
            xp_bias(r_cur, brz8, indr, range(2))
            xp_bias(z_cur, brz8, indz, range(2))
            xp_bias(nx_cur, bnx8, indnx, range(4))
            xp_gemm(0, r_cur, z_cur, nx_cur, range(12))

            for j in range(nblk):
                rp, zp, nxp = r_cur, z_cur, nx_cur
                for k in range(C):
                    t = j * C + k
                    h_rd = hslot(t)
                    last_k = (k == C - 1)

                    # burst: r tiles, hn tiles, z tiles.  With separate
                    # r/z PSUM tiles the hazards are precise: sig_r fires
                    # after the 16 r matmuls retire (~550ns into the burst),
                    # t1 after the hn matmuls (~1100ns), sig_z after the z
                    # matmuls (~1650ns, overlapping the tanh path).
                    for mi in range(4):
                        for kc in range(4):
                            nc.tensor.matmul(
                                rp[:, mi * 64 + k * 8:mi * 64 + k * 8 + 8],
                                whh[:, (kc * 12 + mi) * 128:(kc * 12 + mi + 1) * 128],
                                h_rd[:, kc * 8:(kc + 1) * 8],
                                start=False, stop=(kc == 3),
                                skip_group_check=True,
                            )
                    for i in range(4):
                        for kc in range(4):
                            nc.tensor.matmul(
                                nxp[:, i * 64 + k * 8:i * 64 + k * 8 + 8],
                                whh[:, (kc * 12 + 8 + i) * 128:(kc * 12 + 9 + i) * 128],
                                h_rd[:, kc * 8:(kc + 1) * 8],
                                start=False, stop=(kc == 3),
                                skip_group_check=True,
                            )
                    for mi in range(4, 8):
                        for kc in range(4):
                            nc.tensor.matmul(
                                zp[:, (mi - 4) * 64 + k * 8:(mi - 4) * 64 + k * 8 + 8],
                                whh[:, (kc * 12 + mi) * 128:(kc * 12 + mi + 1) * 128],
                                h_rd[:, kc * 8:(kc + 1) * 8],
                                start=False, stop=(kc == 3),
                                skip_group_check=True,
                            )

                    # TE-only fc work, pre-chain (see fc_mm comment)
                    if j >= 1:
                        if k == 5:
                            fc_mm(j - 1)
                        if k == 6:
                            fc_tr(j - 1)

                    # ---- n-path chain ----
                    rza = work.tile([128, 32], f32, tag="rza")
                    nc.scalar.activation(ap2d(rza, 4, 8), kslice(rp[:], 0, k, 4), Sig)
                    t1 = work.tile([128, 32], f32, tag="t1")
                    nc.vector.tensor_mul(
                        ap2d(t1, 4, 8), ap2d(rza, 4, 8), kslice(nxp[:], 0, k, 4)
                    )
                    t2 = work.tile([128, 32], f32, tag="t2")
                    nc.vector.tensor_add(
                        ap2d(t2, 4, 8), ap2d(t1, 4, 8), kslice(nxp[:], 256, k, 4)
                    )
                    n_t = work.tile([128, 32], f32, tag="n")
                    nc.scalar.activation(n_t[:], t2[:], Tanh)

                    # ---- h update: h' = n + z*(h - n) ----
                    # demote sig_z so the scheduler keeps tanh (critical
                    # path) ahead of it on the single Scalar engine queue
                    zt = work.tile([128, 32], f32, tag="zt")
                    with tc.high_priority(offset=-8):
                        nc.scalar.activation(
                            ap2d(zt, 4, 8), kslice(zp[:], 0, k, 4), Sig
                        )
                    d_t = work.tile([128, 32], f32, tag="d")
                    nc.vector.tensor_sub(d_t[:], h_rd, n_t[:])
                    v_t = work.tile([128, 32], f32, tag="v")
                    nc.vector.tensor_mul(v_t[:], zt[:], d_t[:])
                    nc.vector.tensor_add(hslot(t + 1), n_t[:], v_t[:])

                    # ---- block-level fillers in the TE-idle tail ----
                    # Emitted late in the block (k4-k6) so the WAR edges on
                    # the next block's buffers (last read by block j-1's
                    # final-step chain) are several steps stale by the time
                    # the tensor engine reaches these at the queue head.
                    if j + 1 < nblk:
                        if k == 4:
                            r_cur = pgates.tile([128, 256], f32, tag="r")
                            z_cur = pgates.tile([128, 256], f32, tag="z")
                            nx_cur = pgates.tile([128, 512], f32, tag="nx")
                            xp_bias(r_cur, brz8, indr, range(2))
                            xp_bias(z_cur, brz8, indz, range(2))
                            xp_bias(nx_cur, bnx8, indnx, range(4))
                        if k == 5:
                            xp_gemm(j + 1, r_cur, z_cur, nx_cur, range(0, 6))
                        if k == 6:
                            xp_gemm(j + 1, r_cur, z_cur, nx_cur, range(6, 12))
                    if j >= 1:
                        if k == 5:
                            fc_stt(j - 1)
                        if k == 6:
                            fc_copy(j - 1)
                        if k == 7:
                            fc_dma(j - 1)

            fc_mm(nblk - 1)
            fc_stt(nblk - 1)
            fc_tr(nblk - 1)
            fc_copy(nblk - 1)
            fc_dma(nblk - 1)

    nc.compile()
    return nc


def _prep_weights(F_mat, W_ih, W_hh, b_ih, b_hh, fc_W, fc_b):
    bf = ml_dtypes.bfloat16
    WihT = np.ascontiguousarray(W_ih.T).astype(bf)
    WhhT = np.empty((128, 48 * 128), bf)
    for kc in range(4):
        for m in range(12):
            blk = W_hh[m * 128:(m + 1) * 128, kc * 128:(kc + 1) * 128]
            WhhT[:, (kc * 12 + m) * 128:(kc * 12 + m) * 128 + 128] = blk.T.astype(bf)
    brz8 = np.zeros((128, 128), np.float32)
    brz8[:8] = (b_ih + b_hh)[:2 * HID].reshape(8, 128)
    brz8 = brz8.astype(bf)
    bnx8 = np.zeros((128, 128), np.float32)
    bnx8[:8] = np.concatenate(
        [b_hh[2 * HID:].reshape(4, 128), b_ih[2 * HID:].reshape(4, 128)], axis=0
    )
    bnx8 = bnx8.astype(bf)
    # tile-major indicator matrices: col = tile*64 + k*8 + b
    cr = np.arange(256)
    indr = np.zeros((128, 256), np.float32)
    indz = np.zeros((128, 256), np.float32)
    for jj in range(4):
        indr[jj] = (cr // 64 == jj)
        indz[4 + jj] = (cr // 64 == jj)
    cn = np.arange(512)
    indnx = np.zeros((128, 512), np.float32)
    for jj in range(4):
        indnx[jj] = (cn < 256) & (cn // 64 == jj)
        indnx[4 + jj] = (cn >= 256) & ((cn - 256) // 64 == jj)
    indr = indr.astype(bf)
    indz = indz.astype(bf)
    indnx = indnx.astype(bf)
    fcWT = np.empty((128, 4 * MST), bf)
    for kc in range(4):
        fcWT[:, kc * MST:(kc + 1) * MST] = fc_W[:, kc * 128:(kc + 1) * 128].T.astype(bf)
    fcb = fc_b.reshape(MST, 1).astype(np.float32)
    return dict(WihT=WihT, WhhT=WhhT, brz8=brz8, bnx8=bnx8,
                indr=indr, indz=indz, indnx=indnx, fcWT=fcWT, fcb=fcb)


def kernel(Y, x0_hat, F_mat, W_ih, W_hh, b_ih, b_hh, fc_W, fc_b):
    from concourse.bass_utils import run_bass_kernel_spmd

    t_steps = Y.shape[1]
    if t_steps not in _compiled:
        _compiled[t_steps] = _build_bass(t_steps)
    nc = _compiled[t_steps]

    w = _prep_weights(F_mat, W_ih, W_hh, b_ih, b_hh, fc_W, fc_b)

    # Host-side prior scan: Xp[b, t] = F^{t+1} x0[b]
    bf = ml_dtypes.bfloat16
    Xp = np.empty((B, t_steps, MST), np.float32)
    X = x0_hat.astype(np.float32)
    FT = F_mat.T.astype(np.float32)
    for t in range(t_steps):
        X = X @ FT
        Xp[:, t] = X

    in_maps = []
    for c in range(NCORES):
        sl = slice(c * BS, (c + 1) * BS)
        # inpT: [128, BS*Tt] bf16, col = b*Tt + t; rows 0:64 = Y^T, 64:128 = Xp^T
        Yc = Y[sl].astype(np.float32)                      # [BS, Tt, 64]
        inpT = np.empty((128, BS * t_steps), bf)
        inpT[0:64] = Yc.transpose(2, 0, 1).reshape(64, -1).astype(bf)
        inpT[64:128] = Xp[sl].transpose(2, 0, 1).reshape(64, -1).astype(bf)
        XpTc = np.ascontiguousarray(
            Xp[sl].transpose(2, 0, 1).reshape(64, -1)
        ).astype(np.float32)
        in_maps.append({"inpT": inpT, "XpT": XpTc, **w})
    trace = os.environ.get("KTRACE") == "1"
    res = run_bass_kernel_spmd(nc, in_maps, list(range(NCORES)), trace=trace)
    global LAST_RESULTS
    LAST_RESULTS = res
    out = np.concatenate([res.results[c]["out"] for c in range(NCORES)], axis=0)
    return out.astype(np.float32)


if __name__ == "__main__":
    rng = np.random.default_rng(0)
    ins = {
        "Y": rng.standard_normal((B, int(os.environ.get("KT", T)), NOBS), dtype=np.float32),
        "x0_hat": rng.standard_normal((B, MST), dtype=np.float32),
        "F_mat": (0.99 * np.linalg.qr(rng.standard_normal((MST, MST)))[0]).astype(np.float32),
        "W_ih": 0.05 * rng.standard_normal((H3, 128), dtype=np.float32),
        "W_hh": 0.05 * rng.standard_normal((H3, HID), dtype=np.float32),
        "b_ih": 0.05 * rng.standard_normal(H3, dtype=np.float32),
        "b_hh": 0.05 * rng.standard_normal(H3, dtype=np.float32),
        "fc_W": 0.05 * rng.standard_normal((MST, HID), dtype=np.float32),
        "fc_b": 0.05 * rng.standard_normal(MST, dtype=np.float32),
    }
    print(kernel(**ins).shape)
